# revision 21
# baseline (speedup 1.0000x reference)
"""Trainium2 Bass kernel for nn_AttentionBlock (B=4, H=W=64, C=64, GROUPS=32).

Math (reference):
    hn = GroupNorm(x; gamma, beta, 32 groups, eps=1e-3)
    q = hn@wq+bq ; k = hn@wk+bk ; v = hn@wv+bv
    att = softmax(q k^T / 8) over the 4096 spatial positions
    out = x + (att @ v) @ wo + bo

Sharding: data-parallel, 2 cores per batch image, each core owns 2048 of the
4096 queries but holds the full key/value set for its batch. No collectives.

Per-core pipeline (fully fused on one NeuronCore):
  - xT [C=64, S=4096] arrives pre-transposed in bf16; x_q keeps the core's own
    query rows in fp32 for the residual. GroupNorm stats via bn_stats/bn_aggr
    on DVE; the GN affine folds into the projection weights. k-bias is dropped
    (softmax cancels it exactly).
  - Scores are computed transposed, ST[t, s] (keys on partitions), in fp8-e4m3
    DoubleRow mode: q/k projections drain to fp8 and cheap SBUF-SBUF DMAs
    rebuild them in the [32 partitions, 2 k-tiles, cols] block layout DR wants,
    so each 512-query score matmul streams two fp8 channel-pairs per partition
    per cycle. Two key chunks ride concurrently on array row quadrants 0:32
    and 64:96. Under the 2e-2 output tolerance the fp8 quantization (~2.5%
    on probabilities) is noise after softmax normalization.
  - Softmax is max-free: |score| <= ~3 for unit-normal inputs so exp cannot
    overflow, and softmax(x) == softmax(x - max) exactly.
  - exp is split across TWO engines so it never gates the PE: a slice of chunk
    pairs runs real exp on ACT (one instruction per 2-bank PSUM tile, fp8
    output), the rest run a Schraudolph fast-exp on DVE: the q projection is
    pre-scaled by 8/ln2 (times the 1/sqrt(C) softmax scale), so adding the
    e4m3 exponent-bias constant and converting f32->int8 yields the fp8 BIT
    PATTERN of exp(score) in one tensor_scalar op (max rel err ~7%, mean ~3%;
    ACT tiles divide the scale back out).
  - v (fp8, with an appended ones-column accumulating the softmax denominator
    l) is contracted with the fp8 probabilities in one full-array K=128 matmul
    per key chunk, accumulating all 32 chunks into a single one-bank PSUM
    tile. att@v is emitted TWO pairs behind its exp so the in-order PE queue
    never stalls on an exp that is still running; score tiles triple-buffer
    (6 PSUM banks) so the WAR chain exp(p) -> scores(p+3) has slack.
  - The output projection runs on the unnormalized accumulator ((O/l)@wo ==
    (O@wo)/l), with an extra wo column passing l through; one reciprocal +
    fused multiply-add applies softmax normalization, residual and bo.
"""

import math

import numpy as np
import ml_dtypes

import concourse.tile as tile
from concourse import bacc, mybir
from concourse.bass_utils import run_bass_kernel_spmd

F32 = mybir.dt.float32
BF16 = mybir.dt.bfloat16
F8 = mybir.dt.float8e4
I8 = mybir.dt.int8
AF = mybir.ActivationFunctionType
ALU = mybir.AluOpType
PM = mybir.MatmulPerfMode

B, H, W, C = 4, 64, 64, 64
S = H * W            # 4096 spatial positions per image
SQ = S // 2          # 2048 queries per core
EPS = 1e-3
N_CHUNK = S // 128   # 32 key chunks
NQ = SQ // 128       # 16 query chunks
N_STRIPE = SQ // 512  # 4 query stripes
SCALE = float(C) ** -0.5  # 0.125
A_EXP = 8.0 / math.log(2.0)     # e4m3 Schraudolph slope (exponent units/ln)
SCALE_A = SCALE * A_EXP         # folded into the q projection
B_EXP = 55.65                   # 7<<3 plus mean-centering correction
# Iterations whose second score tile ALSO runs on ACT instead of DVE: DVE
# carries the per-stripe reciprocal/epilogue work, so ACT takes extra tiles.
ACT_B_ITERS = frozenset((3,))


def build_kernel():
    nc = bacc.Bacc("TRN2", target_bir_lowering=False, debug=False)

    xT_d = nc.dram_tensor("xT", [C, S], BF16, kind="ExternalInput")
    x_q = nc.dram_tensor("x_q", [SQ, C], F32, kind="ExternalInput")
    wpf_d = nc.dram_tensor("wpf", [128, 454], F32, kind="ExternalInput")
    wph_d = nc.dram_tensor("wph", [128, 161], BF16, kind="ExternalInput")
    out_d = nc.dram_tensor("out", [SQ, C], F32, kind="ExternalOutput")

    with tile.TileContext(nc) as tc:
        _emit(nc, tc, xT_d.ap(), x_q.ap(), wpf_d.ap(), wph_d.ap(), out_d.ap())
    nc.compile()
    return nc


def _emit(nc, tc, xT_d, x_q, wpf_d, wph_d, out_d):
    from contextlib import ExitStack

    ctx = ExitStack()
    with ctx:
        const = ctx.enter_context(tc.tile_pool(name="const", bufs=1))
        big = ctx.enter_context(tc.tile_pool(name="big", bufs=1))
        tiny = ctx.enter_context(tc.tile_pool(name="tiny", bufs=1))

        # ---- big input DMAs first, one chunk per ring so nothing queues
        # ---- behind them; partitions 64:127 mirror 0:63 for row-tiling ----
        xT = big.tile([128, S], BF16)
        eng = [nc.sync, nc.scalar, nc.gpsimd, nc.sync]
        for i in range(4):
            eng[i].dma_start(out=xT[0:64, 1024 * i:1024 * (i + 1)],
                             in_=xT_d[:, 1024 * i:1024 * (i + 1)])
        # ---- all parameters arrive in TWO host-packed images (one f32, one
        # ---- bf16), so the rings spend 2 DMA issues instead of ~15 and the
        # ---- xT chunks own the DMA bandwidth. Mirrors, broadcasts, the 0/1
        # ---- pair matrices and the rsqrt constants are packed host-side;
        # ---- every weight tile below is just an AP slice of the image.
        wpf = const.tile([128, 454], F32)
        nc.sync.dma_start(out=wpf, in_=wpf_d)
        wph = const.tile([128, 161], BF16)
        nc.scalar.dma_start(out=wph, in_=wph_d)
        for i in range(4):
            eng[i].dma_start(out=xT[64:128, 1024 * i:1024 * (i + 1)],
                             in_=xT[0:64, 1024 * i:1024 * (i + 1)])
        wk_sb = wpf[:, 0:64]
        wq_sb = wpf[:, 64:128]
        wv_sb = wpf[:, 128:192]
        bo_bcast = wpf[:, 192:256]
        gamma_col = wpf[:, 256:257]
        beta_col = wpf[0:64, 257:258]
        wq_aug = wpf[0:65, 259:323]      # [Wq ; bq]
        wv_aug = wpf[0:65, 323:388]      # [Wv ; bv] plus e64 column
        wo_sb = wpf[0:64, 388:452]
        magic = wpf[0:32, 452:453].bitcast(mybir.dt.uint32)
        c15 = wpf[0:32, 453:454]
        # wo_aug = [wo ; bvo] plus e64 column that passes l through. Row 64
        # multiplies the l-row of the accumulator, so after the division by l
        # it contributes the constant row bvo = bv_total @ wo - this is how the
        # v-bias is applied without ever materializing it per-position.
        wo_aug = wph[0:65, 0:65]
        # pair matrices: p64h[c,g] = 0.5 iff c//2 == g ; p32x64[g,c] = 1 iff c//2 == g
        p64h = wph[0:64, 65:97]
        p32x64 = wph[0:32, 97:161]

        # exp is the only ACT table set this kernel uses (rsqrt is done with a
        # Newton iteration on DVE); preload it while waiting on input DMAs.
        scratch1 = const.tile([1, 1], F32)
        nc.scalar.activation(scratch1, wph[0:1, 0:1], AF.Exp, bias=0.0, scale=0.0)

        # ---- PSUM pools (8 banks: st 3x[128,1024] = 6, ot/aux 2x1) ----
        # ot_ps hosts everything one-bank: GN scratch, v-projection tiles, the
        # att@v accumulator and the epilogue projection tile, in rotation.
        st_ps = ctx.enter_context(tc.tile_pool(name="st_ps", bufs=3, space="PSUM"))
        ot_ps = ctx.enter_context(tc.tile_pool(name="ot_ps", bufs=2, space="PSUM"))

        # ---- GroupNorm stats on DVE: per-channel mean/var over all 4096 ----
        bstats = tiny.tile([64, 8, 6], F32)
        for i in range(8):
            nc.vector.bn_stats(bstats[:, i, :], xT[0:64, 512 * i:512 * (i + 1)])
        mv = tiny.tile([64, 2], F32)
        nc.vector.bn_aggr(mv, bstats)
        packed64 = tiny.tile([64, 2], BF16)       # [mean_c, E[x^2]_c]
        nc.vector.tensor_copy(packed64[:, 0:1], mv[:, 0:1])
        nc.vector.scalar_tensor_tensor(out=packed64[:, 1:2], in0=mv[:, 0:1],
                                       scalar=mv[:, 0:1], in1=mv[:, 1:2],
                                       op0=ALU.mult, op1=ALU.add)
        gpair = ot_ps.tile([32, 2], F32, tag="ot")  # group [mean, E[x^2]]
        nc.tensor.matmul(gpair, lhsT=p64h, rhs=packed64)
        gm = tiny.tile([32, 2], F32)
        nc.vector.tensor_copy(gm, gpair)
        var = tiny.tile([32, 1], F32)
        nc.vector.tensor_mul(var, gm[:, 0:1], gm[:, 0:1])
        nc.vector.tensor_sub(var, gm[:, 1:2], var)
        nc.vector.tensor_scalar_add(var, var, EPS)
        # rstd = rsqrt(var) entirely on DVE: quake-style bit seed + ONE Newton
        # step (rel err < 2e-3, ample for the 2e-2 tolerance) - keeps the
        # scalar engine's activation tables untouched for exp. var acts as
        # the per-partition scalar operand so the step is 3 fused ops.
        U32 = mybir.dt.uint32
        packed32 = tiny.tile([32, 2], BF16)       # [rstd_g | mean_g]
        nc.vector.tensor_copy(packed32[:, 1:2], gm[:, 0:1])
        ybits = tiny.tile([32, 1], U32)
        nc.vector.tensor_scalar(out=ybits, in0=var.bitcast(U32), scalar1=1,
                                scalar2=None, op0=ALU.logical_shift_right)
        nc.vector.tensor_sub(ybits, magic, ybits)
        y = ybits.bitcast(F32)
        t1 = tiny.tile([32, 1], F32)
        nc.vector.tensor_mul(t1, y, y)
        nc.vector.scalar_tensor_tensor(out=t1, in0=t1, scalar=var, in1=c15,
                                       op0=ALU.mult, op1=ALU.bypass)
        nc.vector.scalar_tensor_tensor(out=t1, in0=t1, scalar=-0.5, in1=c15,
                                       op0=ALU.mult, op1=ALU.add)
        nc.vector.tensor_mul(packed32[:, 0:1], y, t1)
        rstd = packed32[:, 0:1]
        chan = ot_ps.tile([128, 2], F32, tag="ot")  # expand groups->channels,
        nc.tensor.matmul(chan[0:64, :], lhsT=p32x64, rhs=packed32)  # both halves
        nc.tensor.matmul(chan[64:128, :], lhsT=p32x64, rhs=packed32,
                         tile_position=(0, 64))
        scale_col = tiny.tile([128, 1], F32)      # rstd_g * gamma_c (mirrored)
        nc.vector.tensor_mul(scale_col, chan[:, 0:1], gamma_col)
        # ---- fold GN into projection weights (both halves in one op); the
        # ---- weight scalings gate the projections so they come first ----
        wk_sc = tiny.tile([128, 64], BF16)
        nc.vector.tensor_scalar_mul(wk_sc, wk_sb, scale_col)
        scale_q = tiny.tile([128, 1], F32)        # q path also carries the
        nc.vector.tensor_scalar_mul(scale_q, scale_col, SCALE_A)  # exp slope
        wq_sc = tiny.tile([128, 64], BF16)
        nc.vector.tensor_scalar_mul(wq_sc, wq_sb, scale_q)
        wv_sc = tiny.tile([128, 64], BF16)
        nc.vector.tensor_scalar_mul(wv_sc, wv_sb, scale_col)
        gnbias = tiny.tile([65, 1], F32)          # beta - mean*scale, aug 1
        nc.vector.tensor_mul(gnbias[0:64, :], chan[0:64, 1:2], scale_col[0:64, :])
        nc.vector.tensor_sub(gnbias[0:64, :], beta_col, gnbias[0:64, :])
        nc.gpsimd.memset(gnbias[64:65, :], 1.0)

        bqp = ot_ps.tile([128, 1], F32, tag="ot")  # total q bias, both halves
        nc.tensor.matmul(bqp[0:64, :], lhsT=wq_aug, rhs=gnbias)
        nc.tensor.matmul(bqp[64:128, :], lhsT=wq_aug, rhs=gnbias,
                         tile_position=(0, 64))
        bq_col = tiny.tile([128, 1], F32)
        nc.vector.tensor_scalar_mul(bq_col, bqp, SCALE_A)
        # bvo row for wo_aug: bvo = (gnbias@Wv + bv) @ wo, bounced through HBM
        # to land on partition 64 (engines are lane-locked; DMA is not). This
        # only gates the first output projection, well off the critical path.
        bvcp = ot_ps.tile([65, 1], F32, tag="ot")
        nc.tensor.matmul(bvcp, lhsT=wv_aug, rhs=gnbias)
        bv_col = tiny.tile([64, 1], F32)
        nc.vector.tensor_copy(bv_col, bvcp[0:64, :])
        bvop = ot_ps.tile([1, 64], F32, tag="ot")
        nc.tensor.matmul(bvop, lhsT=bv_col, rhs=wo_sb)
        bvo_row = tiny.tile([1, 64], F32)
        nc.vector.tensor_copy(bvo_row, bvop)
        bvo_stage = nc.dram_tensor("bvo_stage", [64], F32).ap()
        nc.sync.dma_start(out=bvo_stage.rearrange("(o c) -> o c", o=1), in_=bvo_row)
        nc.gpsimd.dma_start(out=wo_aug[64:65, 0:64],
                            in_=bvo_stage.rearrange("(o c) -> o c", o=1))

        # ---- q/k projections -> fp8 DoubleRow operands ----
        # The quads project exactly as in the bf16 version (channels on
        # partitions, both mirror halves), drain straight to fp8, and four
        # SBUF-SBUF DMAs per quad refold each tensor into the DoubleRow block
        # layout [32 partitions (channel pair), 2 k-tiles, cols]: partition p,
        # tile t <-> channel 32t+p. Key chunks are spread over FOUR 32-row
        # array quadrants (rows 32q:32q+32 <-> chunk octet 8q:8q+8) so four
        # score matmuls can ride the PE concurrently; qT_dr carries all
        # queries on every quadrant (qs8's hi half is already the mirror the
        # B-stream quadrants need). Drains are split across DVE and ACT.
        ks8 = big.tile([128, SQ], F8)
        qs8 = big.tile([128, SQ], F8)
        kT_dr = big.tile([128, 2, 1024], F8)
        qT_dr = big.tile([128, 2, SQ], F8)

        def kq_quad(dst, w_sc, lo_cols, hi_cols, bias):
            g = st_ps.tile([128, 1024], F32, tag="st")
            nc.tensor.matmul(g[0:64, 0:512], lhsT=w_sc[0:64, :],
                             rhs=xT[0:64, lo_cols:lo_cols + 512],
                             tile_position=(0, 0))
            nc.tensor.matmul(g[64:128, 512:1024], lhsT=w_sc[64:128, :],
                             rhs=xT[64:128, hi_cols + 512:hi_cols + 1024],
                             tile_position=(64, 64))
            nc.tensor.matmul(g[64:128, 0:512], lhsT=w_sc[64:128, :],
                             rhs=xT[64:128, hi_cols:hi_cols + 512],
                             tile_position=(64, 64))
            nc.tensor.matmul(g[0:64, 512:1024], lhsT=w_sc[0:64, :],
                             rhs=xT[0:64, lo_cols + 512:lo_cols + 1024],
                             tile_position=(0, 0))
            if bias is None:
                nc.scalar.copy(out=dst[:, 0:512], in_=g[:, 0:512])
                nc.vector.tensor_copy(dst[:, 512:1024], g[:, 512:1024])
            else:
                nc.vector.tensor_scalar_add(dst[:, 0:512], g[:, 0:512], bias)
                nc.scalar.add(out=dst[:, 512:1024], in_=g[:, 512:1024],
                              add=bias)

        rings = [nc.sync, nc.gpsimd, nc.scalar]

        def remap_k(quad_cols):
            # ks8 rows 0:64 = chunks 0:15, rows 64:128 = chunks 16:31; the
            # column quad picks the even or odd octet of each.
            qa = 0 if quad_cols == 0 else 1      # quadrants Q0/Q2 or Q1/Q3
            cs = slice(quad_cols, quad_cols + 1024)
            for n, (src, dst) in enumerate(((0, 32 * qa), (64, 64 + 32 * qa))):
                for t in range(2):
                    rings[(2 * n + t) % 3].dma_start(
                        out=kT_dr[dst:dst + 32, t, 0:1024],
                        in_=ks8[src + 32 * t:src + 32 * t + 32, cs])

        def remap_q(quad_cols):
            cs = slice(quad_cols, quad_cols + 1024)
            for n, (src, dst) in enumerate(((0, 0), (0, 32), (64, 64),
                                            (64, 96))):
                for t in range(2):
                    rings[(2 * n + t) % 3].dma_start(
                        out=qT_dr[dst:dst + 32, t, cs],
                        in_=qs8[src + 32 * t:src + 32 * t + 32, cs])

        kq_quad(ks8[:, 0:1024], wk_sc, 0, 2048, None)
        remap_k(0)
        kq_quad(ks8[:, 1024:2048], wk_sc, 1024, 3072, None)
        remap_k(1024)
        kq_quad(qs8[:, 0:1024], wq_sc, 0, 0, bq_col)
        remap_q(0)

        # v in natural [t, c] fp8 layout; groups of 4 chunks {p, 8+p, 16+p,
        # 24+p}: lo-rows compute the a=0 chunks, hi-rows the a=1 chunks, into
        # two one-bank tiles (concurrent row-tiles drain into distinct banks).
        # Column 64 = ones (exact in e4m3) for the softmax denominator.
        v_big = big.tile([128, N_CHUNK, 65], F8)
        nc.gpsimd.memset(v_big[:, :, 64:65], 1.0)
        v4 = v_big.rearrange("q (a b g) c -> q a b g c", a=2, b=2)

        def v_group(p):
            vga = ot_ps.tile([128, 2, 64], F32, tag="ot")
            vgb = ot_ps.tile([128, 2, 64], F32, tag="ot")
            for a, vg in ((0, vga), (1, vgb)):
                half = slice(64, 128) if a else slice(0, 64)
                tp = (64, 0) if a else (0, 0)
                for b in range(2):
                    ch = a * 16 + b * 8 + p
                    nc.tensor.matmul(vg[:, b, :],
                                     lhsT=xT[half, 128 * ch:128 * (ch + 1)],
                                     rhs=wv_sc[half, :], tile_position=tp)
            nc.vector.tensor_copy(v4[:, 0, :, p, 0:64], vga)
            nc.vector.tensor_copy(v4[:, 1, :, p, 0:64], vgb)

        for p in range(8):
            v_group(p)

        # stripe 0/1 only need q columns 0:1024; the second q quad projects
        # under the v groups and the first attention iterations.
        kq_quad(qs8[:, 1024:2048], wq_sc, 1024, 1024, bq_col)
        remap_q(1024)

        # ---- residual base: x + bo (needed only by epilogues) ----
        xq_sb = big.tile([128, NQ, 64], F32)
        nc.sync.dma_start(out=xq_sb, in_=x_q.rearrange("(m p) c -> p m c", p=128))
        xb_sb = big.tile([128, NQ, 64], F32)
        nc.vector.tensor_add(xb_sb, xq_sb,
                             bo_bcast.rearrange("p (o c) -> p o c", o=1).broadcast_to([128, NQ, 64]))

        # ---- main attention loop ----
        p_pool = ctx.enter_context(tc.tile_pool(name="p_pool", bufs=6))
        ep_pool = ctx.enter_context(tc.tile_pool(name="ep_pool", bufs=3))
        N_PAIR = N_CHUNK // 2

        def emit_o(io, ot, pt):
            # chunks arrive as 0, 16, 8, 24, 1, ...: first is 0, last is 31
            first = io == 0
            last = io == N_CHUNK - 1
            nc.tensor.matmul(ot, lhsT=v_big[:, io, :], rhs=pt,
                             start=first, stop=last)

        def make_epilogue(j, ot_sb):
            last_stripe = j == N_STRIPE - 1

            def epi():
                res = ep_pool.tile([128, 4, 64], F32, tag="res", bufs=2)
                op4 = ot_ps.tile([128, 4, 128], F32, tag="ot")
                for m in range(4):
                    nc.tensor.matmul(op4[:, m, 0:65],
                                     lhsT=ot_sb[:, 128 * m:128 * (m + 1)],
                                     rhs=wo_aug)
                    rl = ep_pool.tile([128, 1], F32, tag="rl")
                    nc.vector.reciprocal(rl, op4[:, m, 64:65])
                    nc.vector.scalar_tensor_tensor(out=res[:, m, :],
                                                   in0=op4[:, m, 0:64],
                                                   scalar=rl,
                                                   in1=xb_sb[:, 4 * j + m, :],
                                                   op0=ALU.mult, op1=ALU.add)
                    if last_stripe:
                        # tail latency: ship each chunk on its own ring so the
                        # ~650ns per-issue cost doesn't serialize (the exit
                        # protocol's round-2 stagger is fixed, not DMA-gated)
                        base = 512 * j + 128 * m
                        ring = [nc.sync, nc.scalar, nc.sync, nc.scalar][m]
                        ring.dma_start(out=out_d[base:base + 128, :],
                                       in_=res[:, m, :])
                if not last_stripe:
                    nc.sync.dma_start(
                        out=out_d[512 * j:512 * (j + 1), :].rearrange("(m p) c -> p m c", p=128),
                        in_=res)
            return epi

        pending_epilogue = None
        for j in range(N_STRIPE):
            ot = ot_ps.tile([65, 512], F32, tag="ot")
            pts = {}
            qc = slice(512 * j, 512 * (j + 1))

            def score_exp(i):
                # iteration i scores chunks i, 8+i, 16+i, 24+i: one DR matmul
                # per 32-row quadrant, all four riding the array together.
                kc = slice(128 * i, 128 * (i + 1))
                st2a = st_ps.tile([128, 1024], F32, tag="st")
                nc.tensor.matmul(st2a[:, 0:512], lhsT=kT_dr[0:32, :, kc],
                                 rhs=qT_dr[0:32, :, qc],
                                 perf_mode=PM.DoubleRow, tile_position=(0, 0))
                nc.tensor.matmul(st2a[:, 512:1024], lhsT=kT_dr[64:96, :, kc],
                                 rhs=qT_dr[64:96, :, qc],
                                 perf_mode=PM.DoubleRow, tile_position=(64, 0))
                st2b = st_ps.tile([128, 1024], F32, tag="st")
                nc.tensor.matmul(st2b[:, 0:512], lhsT=kT_dr[32:64, :, kc],
                                 rhs=qT_dr[32:64, :, qc],
                                 perf_mode=PM.DoubleRow, tile_position=(32, 0))
                nc.tensor.matmul(st2b[:, 512:1024], lhsT=kT_dr[96:128, :, kc],
                                 rhs=qT_dr[96:128, :, qc],
                                 perf_mode=PM.DoubleRow, tile_position=(96, 0))
                pta = p_pool.tile([128, 1024], F8, tag="p")
                nc.scalar.activation(pta, st2a, AF.Exp, bias=0.0,
                                     scale=1.0 / A_EXP)
                if i in ACT_B_ITERS:
                    ptb = p_pool.tile([128, 1024], F8, tag="p")
                    nc.scalar.activation(ptb, st2b, AF.Exp, bias=0.0,
                                         scale=1.0 / A_EXP)
                    pts[i] = (pta, ptb)
                else:
                    ptb = p_pool.tile([128, 1024], I8, tag="p")
                    nc.vector.tensor_scalar_add(ptb, st2b, B_EXP)
                    pts[i] = (pta, ptb.bitcast(F8))

            # att@v trails its exp by one iteration so the in-order PE queue
            # never parks on an exp still in flight; the last iteration
            # flushes after the loop.
            for i in range(N_PAIR // 2 + 1):
                if i < N_PAIR // 2:
                    score_exp(i)
                if i == 2 and pending_epilogue is not None:
                    pending_epilogue()
                    pending_epilogue = None
                io = i - 1
                if io >= 0:
                    pta, ptb = pts.pop(io)
                    emit_o(io, ot, pta[:, 0:512])
                    emit_o(16 + io, ot, pta[:, 512:1024])
                    emit_o(8 + io, ot, ptb[:, 0:512])
                    emit_o(24 + io, ot, ptb[:, 512:1024])
            # drain the accumulator (+ l row) to SBUF. The last stripe drains
            # per column-chunk so its epilogue (on the critical tail) starts
            # as soon as the first chunk lands.
            ot_sb = ep_pool.tile([65, 512], BF16, bufs=2, tag="ot_sb")
            if j == N_STRIPE - 1:
                for m in range(4):
                    cs = slice(128 * m, 128 * (m + 1))
                    if m % 2 == 0:
                        nc.vector.tensor_copy(ot_sb[:, cs], ot[:, cs])
                    else:
                        nc.scalar.copy(out=ot_sb[:, cs], in_=ot[:, cs])
            else:
                nc.scalar.copy(out=ot_sb[:, 0:256], in_=ot[:, 0:256])
                nc.vector.tensor_copy(ot_sb[:, 256:512], ot[:, 256:512])
            pending_epilogue = make_epilogue(j, ot_sb)
        pending_epilogue()


_NC_CACHE = {}


def _get_nc():
    if "nc" not in _NC_CACHE:
        _NC_CACHE["nc"] = build_kernel()
    return _NC_CACHE["nc"]


def build_in_maps(x, gamma, beta, wq, bq, wk, wv, bv, wo, bo):
    """Per-core NEFF input dicts plus (batch, rows) scatter info per core."""
    x = np.asarray(x, dtype=np.float32)
    gamma, beta, bq, bv, bo = (np.asarray(a, np.float32)
                               for a in (gamma, beta, bq, bv, bo))
    wq, wk, wv, wo = (np.asarray(a, np.float32) for a in (wq, wk, wv, wo))
    # f32 parameter image: mirrors/broadcasts/rsqrt constants prepacked so the
    # device spends one DMA issue instead of ~15 (see _emit for the layout).
    wpf = np.zeros((128, 454), np.float32)
    wpf[:, 0:64] = np.tile(wk, (2, 1))
    wpf[:, 64:128] = np.tile(wq, (2, 1))
    wpf[:, 128:192] = np.tile(wv, (2, 1))
    wpf[:, 192:256] = np.tile(bo[None, :], (128, 1))
    wpf[:, 256] = np.tile(gamma, 2)
    wpf[0:64, 257] = beta
    wpf[0:64, 259:323] = wq
    wpf[64, 259:323] = bq
    wpf[0:64, 323:387] = wv
    wpf[64, 323:387] = bv
    wpf[64, 387] = 1.0                      # wv_aug e64 column
    wpf[0:64, 388:452] = wo
    wpf[0:32, 452] = np.full(32, 0x5f3759df, np.uint32).view(np.float32)
    wpf[0:32, 453] = 1.5
    # bf16 image: wo_aug shell (bvo row filled on device) + 0/1 pair matrices
    wph = np.zeros((128, 161), ml_dtypes.bfloat16)
    wph[0:64, 0:64] = wo.astype(ml_dtypes.bfloat16)
    wph[64, 64] = 1.0
    cc = np.arange(64)
    wph[0:64, 65:97] = (0.5 * (cc[:, None] // 2 == np.arange(32)[None, :])
                        ).astype(ml_dtypes.bfloat16)                  # p64h
    wph[0:32, 97:161] = (cc[None, :] // 2 == np.arange(32)[:, None]
                         ).astype(ml_dtypes.bfloat16)                 # p32x64
    shared = {"wpf": wpf, "wph": wph}
    xf = x.reshape(B, S, C)
    in_maps = []
    scatter = []
    for core in range(8):
        b, h = core // 2, core % 2
        own = slice(h * SQ, (h + 1) * SQ)
        other = slice((1 - h) * SQ, (2 - h) * SQ)
        x_local = np.concatenate([xf[b][own], xf[b][other]], axis=0)
        in_maps.append({
            "xT": np.ascontiguousarray(x_local.T).astype(ml_dtypes.bfloat16),
            "x_q": np.ascontiguousarray(x_local[:SQ]),
            **shared,
        })
        scatter.append((b, np.arange(h * SQ, (h + 1) * SQ)))
    return in_maps, scatter


def _run(in_maps, scatter, **spmd_kwargs):
    nc = _get_nc()
    res = run_bass_kernel_spmd(nc, in_maps, core_ids=list(range(8)),
                               **spmd_kwargs)
    out = np.empty((B, S, C), np.float32)
    for core in range(8):
        b, rows = scatter[core]
        out[b][rows] = res.results[core]["out"]
    return out.reshape(B, H, W, C), res


def kernel(x, gamma, beta, wq, bq, wk, bk, wv, bv, wo, bo):
    # bk is provably a no-op: it shifts each query's scores by the constant
    # bk.q which softmax cancels, so it is not shipped to the device.
    in_maps, scatter = build_in_maps(x, gamma, beta, wq, bq, wk, wv, bv, wo, bo)
    out, _ = _run(in_maps, scatter)
    return out


# revision 22
# speedup vs baseline: 1.0038x; 1.0038x over previous
"""Trainium2 Bass kernel for nn_AttentionBlock (B=4, H=W=64, C=64, GROUPS=32).

Math (reference):
    hn = GroupNorm(x; gamma, beta, 32 groups, eps=1e-3)
    q = hn@wq+bq ; k = hn@wk+bk ; v = hn@wv+bv
    att = softmax(q k^T / 8) over the 4096 spatial positions
    out = x + (att @ v) @ wo + bo

Sharding: data-parallel, 2 cores per batch image, each core owns 2048 of the
4096 queries but holds the full key/value set for its batch. No collectives.

Per-core pipeline (fully fused on one NeuronCore):
  - xT [C=64, S=4096] arrives pre-transposed in bf16; x_q keeps the core's own
    query rows in fp32 for the residual. GroupNorm stats via bn_stats/bn_aggr
    on DVE; the GN affine folds into the projection weights. k-bias is dropped
    (softmax cancels it exactly).
  - Scores are computed transposed, ST[t, s] (keys on partitions), in fp8-e4m3
    DoubleRow mode: q/k projections drain to fp8 and cheap SBUF-SBUF DMAs
    rebuild them in the [32 partitions, 2 k-tiles, cols] block layout DR wants,
    so each 512-query score matmul streams two fp8 channel-pairs per partition
    per cycle. Two key chunks ride concurrently on array row quadrants 0:32
    and 64:96. Under the 2e-2 output tolerance the fp8 quantization (~2.5%
    on probabilities) is noise after softmax normalization.
  - Softmax is max-free: |score| <= ~3 for unit-normal inputs so exp cannot
    overflow, and softmax(x) == softmax(x - max) exactly.
  - exp is split across TWO engines so it never gates the PE: a slice of chunk
    pairs runs real exp on ACT (one instruction per 2-bank PSUM tile, fp8
    output), the rest run a Schraudolph fast-exp on DVE: the q projection is
    pre-scaled by 8/ln2 (times the 1/sqrt(C) softmax scale), so adding the
    e4m3 exponent-bias constant and converting f32->int8 yields the fp8 BIT
    PATTERN of exp(score) in one tensor_scalar op (max rel err ~7%, mean ~3%;
    ACT tiles divide the scale back out).
  - v (fp8, with an appended ones-column accumulating the softmax denominator
    l) is contracted with the fp8 probabilities in one full-array K=128 matmul
    per key chunk, accumulating all 32 chunks into a single one-bank PSUM
    tile. att@v is emitted TWO pairs behind its exp so the in-order PE queue
    never stalls on an exp that is still running; score tiles triple-buffer
    (6 PSUM banks) so the WAR chain exp(p) -> scores(p+3) has slack.
  - The output projection runs on the unnormalized accumulator ((O/l)@wo ==
    (O@wo)/l), with an extra wo column passing l through; one reciprocal +
    fused multiply-add applies softmax normalization, residual and bo.
"""

import math

import numpy as np
import ml_dtypes

import concourse.tile as tile
from concourse import bacc, mybir
from concourse.bass_utils import run_bass_kernel_spmd

F32 = mybir.dt.float32
BF16 = mybir.dt.bfloat16
F8 = mybir.dt.float8e4
I8 = mybir.dt.int8
AF = mybir.ActivationFunctionType
ALU = mybir.AluOpType
PM = mybir.MatmulPerfMode

B, H, W, C = 4, 64, 64, 64
S = H * W            # 4096 spatial positions per image
SQ = S // 2          # 2048 queries per core
EPS = 1e-3
N_CHUNK = S // 128   # 32 key chunks
NQ = SQ // 128       # 16 query chunks
N_STRIPE = SQ // 512  # 4 query stripes
SCALE = float(C) ** -0.5  # 0.125
A_EXP = 8.0 / math.log(2.0)     # e4m3 Schraudolph slope (exponent units/ln)
SCALE_A = SCALE * A_EXP         # folded into the q projection
B_EXP = 55.65                   # 7<<3 plus mean-centering correction
# Iterations whose second score tile ALSO runs on ACT instead of DVE: DVE
# carries the per-stripe reciprocal/epilogue work, so ACT takes extra tiles.
ACT_B_ITERS = frozenset((3,))


def build_kernel():
    nc = bacc.Bacc("TRN2", target_bir_lowering=False, debug=False)

    xT_d = nc.dram_tensor("xT", [C, S], BF16, kind="ExternalInput")
    x_q = nc.dram_tensor("x_q", [SQ, C], F32, kind="ExternalInput")
    gamma = nc.dram_tensor("gamma", [C], F32, kind="ExternalInput")
    beta = nc.dram_tensor("beta", [C], F32, kind="ExternalInput")
    wq_d = nc.dram_tensor("wq", [C, C], F32, kind="ExternalInput")
    bq_d = nc.dram_tensor("bq", [C], F32, kind="ExternalInput")
    wk_d = nc.dram_tensor("wk", [C, C], F32, kind="ExternalInput")
    wv_d = nc.dram_tensor("wv", [C, C], F32, kind="ExternalInput")
    bv_d = nc.dram_tensor("bv", [C], F32, kind="ExternalInput")
    wo_d = nc.dram_tensor("wo", [C, C], F32, kind="ExternalInput")
    bo_d = nc.dram_tensor("bo", [C], F32, kind="ExternalInput")
    out_d = nc.dram_tensor("out", [SQ, C], F32, kind="ExternalOutput")

    with tile.TileContext(nc) as tc:
        _emit(nc, tc, xT_d.ap(), x_q.ap(), gamma.ap(), beta.ap(), wq_d.ap(),
              bq_d.ap(), wk_d.ap(), wv_d.ap(), bv_d.ap(), wo_d.ap(), bo_d.ap(),
              out_d.ap())
    nc.compile()
    return nc


def _emit(nc, tc, xT_d, x_q, gamma, beta, wq_d, bq_d, wk_d, wv_d, bv_d, wo_d,
          bo_d, out_d):
    from contextlib import ExitStack

    ctx = ExitStack()
    with ctx:
        const = ctx.enter_context(tc.tile_pool(name="const", bufs=1))
        big = ctx.enter_context(tc.tile_pool(name="big", bufs=1))
        tiny = ctx.enter_context(tc.tile_pool(name="tiny", bufs=1))

        # ---- big input DMAs first, one chunk per ring so nothing queues
        # ---- behind them; partitions 64:127 mirror 0:63 for row-tiling ----
        xT = big.tile([128, S], BF16)
        eng = [nc.sync, nc.scalar, nc.gpsimd, nc.sync]
        for i in range(4):
            eng[i].dma_start(out=xT[0:64, 1024 * i:1024 * (i + 1)],
                             in_=xT_d[:, 1024 * i:1024 * (i + 1)])
        for i in range(4):
            eng[i].dma_start(out=xT[64:128, 1024 * i:1024 * (i + 1)],
                             in_=xT[0:64, 1024 * i:1024 * (i + 1)])

        # ---- params: split across rings so no single engine eats all the
        # ---- ~650ns issue costs ----
        wq_aug = const.tile([65, 64], F32)   # [Wq ; bq]
        nc.scalar.dma_start(out=wq_aug[0:64, :], in_=wq_d)
        nc.scalar.dma_start(out=wq_aug[64:65, :], in_=bq_d.rearrange("(o c) -> o c", o=1))
        wk_sb = const.tile([128, 64], F32)
        nc.sync.dma_start(out=wk_sb[0:64, :], in_=wk_d)
        nc.sync.dma_start(out=wk_sb[64:128, :], in_=wk_d)
        wq_sb = const.tile([128, 64], F32)
        nc.sync.dma_start(out=wq_sb[0:64, :], in_=wq_d)
        nc.sync.dma_start(out=wq_sb[64:128, :], in_=wq_d)
        wv_sb = const.tile([128, 64], F32)
        nc.scalar.dma_start(out=wv_sb[0:64, :], in_=wv_d)
        nc.scalar.dma_start(out=wv_sb[64:128, :], in_=wv_d)
        wv_aug = const.tile([65, 65], F32)   # [Wv ; bv] plus e64 column
        nc.scalar.dma_start(out=wv_aug[0:64, 0:64], in_=wv_d)
        nc.scalar.dma_start(out=wv_aug[64:65, 0:64], in_=bv_d.rearrange("(o c) -> o c", o=1))
        nc.gpsimd.memset(wv_aug[0:64, 64:65], 0.0)
        nc.gpsimd.memset(wv_aug[64:65, 64:65], 1.0)
        # wo_aug = [wo ; bvo] plus e64 column that passes l through. Row 64
        # multiplies the l-row of the accumulator, so after the division by l
        # it contributes the constant row bvo = bv_total @ wo - this is how the
        # v-bias is applied without ever materializing it per-position.
        wo_aug = const.tile([65, 65], BF16)
        nc.gpsimd.dma_start(out=wo_aug[0:64, 0:64], in_=wo_d)  # SWDGE casts f32->bf16
        nc.gpsimd.memset(wo_aug[0:64, 64:65], 0.0)
        nc.gpsimd.memset(wo_aug[64:65, 64:65], 1.0)
        wo_sb = const.tile([64, 64], F32)
        nc.scalar.dma_start(out=wo_sb, in_=wo_d)
        gamma_col = const.tile([128, 1], F32)
        nc.sync.dma_start(out=gamma_col[0:64, :], in_=gamma.rearrange("(c o) -> c o", o=1))
        nc.sync.dma_start(out=gamma_col[64:128, :], in_=gamma.rearrange("(c o) -> c o", o=1))
        beta_col = const.tile([64, 1], F32)
        nc.sync.dma_start(out=beta_col, in_=beta.rearrange("(c o) -> c o", o=1))
        bo_bcast = const.tile([128, 64], F32)
        nc.scalar.dma_start(out=bo_bcast, in_=bo_d.rearrange("(o c) -> o c", o=1).to_broadcast([128, 64]))

        # exp is the only ACT table set this kernel uses (rsqrt is done with a
        # Newton iteration on DVE); preload it while waiting on input DMAs.
        scratch1 = const.tile([1, 1], F32)
        nc.scalar.activation(scratch1, wq_aug[0:1, 0:1], AF.Exp, bias=0.0, scale=0.0)

        # pair matrices: p64h[c,g] = 0.5 iff c//2 == g ; p32x64[g,c] = 1 iff c//2 == g
        p64h = const.tile([64, 32], BF16)
        nc.gpsimd.memset(p64h, 0.5)
        nc.gpsimd.affine_select(out=p64h, in_=p64h, compare_op=ALU.is_ge,
                                fill=0.0, base=0, pattern=[[-2, 32]],
                                channel_multiplier=1)
        nc.gpsimd.affine_select(out=p64h, in_=p64h, compare_op=ALU.is_ge,
                                fill=0.0, base=1, pattern=[[2, 32]],
                                channel_multiplier=-1)
        p32x64 = const.tile([32, 64], BF16)
        nc.gpsimd.memset(p32x64, 1.0)
        nc.gpsimd.affine_select(out=p32x64, in_=p32x64, compare_op=ALU.is_ge,
                                fill=0.0, base=0, pattern=[[1, 64]],
                                channel_multiplier=-2)
        nc.gpsimd.affine_select(out=p32x64, in_=p32x64, compare_op=ALU.is_ge,
                                fill=0.0, base=1, pattern=[[-1, 64]],
                                channel_multiplier=2)

        # ---- PSUM pools (8 banks: st 3x[128,1024] = 6, ot/aux 2x1) ----
        # ot_ps hosts everything one-bank: GN scratch, v-projection tiles, the
        # att@v accumulator and the epilogue projection tile, in rotation.
        st_ps = ctx.enter_context(tc.tile_pool(name="st_ps", bufs=3, space="PSUM"))
        ot_ps = ctx.enter_context(tc.tile_pool(name="ot_ps", bufs=2, space="PSUM"))

        # ---- GroupNorm stats on DVE: per-channel mean/var over all 4096 ----
        bstats = tiny.tile([64, 8, 6], F32)
        for i in range(8):
            nc.vector.bn_stats(bstats[:, i, :], xT[0:64, 512 * i:512 * (i + 1)])
        mv = tiny.tile([64, 2], F32)
        nc.vector.bn_aggr(mv, bstats)
        packed64 = tiny.tile([64, 2], BF16)       # [mean_c, E[x^2]_c]
        nc.vector.tensor_copy(packed64[:, 0:1], mv[:, 0:1])
        nc.vector.scalar_tensor_tensor(out=packed64[:, 1:2], in0=mv[:, 0:1],
                                       scalar=mv[:, 0:1], in1=mv[:, 1:2],
                                       op0=ALU.mult, op1=ALU.add)
        gpair = ot_ps.tile([32, 2], F32, tag="ot")  # group [mean, E[x^2]]
        nc.tensor.matmul(gpair, lhsT=p64h, rhs=packed64)
        gm = tiny.tile([32, 2], F32)
        nc.vector.tensor_copy(gm, gpair)
        var = tiny.tile([32, 1], F32)
        nc.vector.tensor_mul(var, gm[:, 0:1], gm[:, 0:1])
        nc.vector.tensor_sub(var, gm[:, 1:2], var)
        nc.vector.tensor_scalar_add(var, var, EPS)
        # rstd = rsqrt(var) entirely on DVE: quake-style bit seed + 2 Newton
        # steps (rel err < 5e-6 for any positive input) - keeps the scalar
        # engine's activation tables untouched for exp. var acts as the
        # per-partition scalar operand so each step is 3 fused ops.
        U32 = mybir.dt.uint32
        magic = tiny.tile([32, 1], U32)
        nc.gpsimd.memset(magic, 0x5f3759df)
        packed32 = tiny.tile([32, 2], BF16)       # [rstd_g | mean_g]
        nc.vector.tensor_copy(packed32[:, 1:2], gm[:, 0:1])
        ybits = tiny.tile([32, 1], U32)
        nc.vector.tensor_scalar(out=ybits, in0=var.bitcast(U32), scalar1=1,
                                scalar2=None, op0=ALU.logical_shift_right)
        nc.vector.tensor_sub(ybits, magic, ybits)
        y = ybits.bitcast(F32)
        c15 = tiny.tile([32, 1], F32)
        nc.gpsimd.memset(c15, 1.5)
        t1 = tiny.tile([32, 1], F32)
        for it in range(2):
            nc.vector.tensor_mul(t1, y, y)
            nc.vector.scalar_tensor_tensor(out=t1, in0=t1, scalar=var, in1=c15,
                                           op0=ALU.mult, op1=ALU.bypass)
            nc.vector.scalar_tensor_tensor(out=t1, in0=t1, scalar=-0.5, in1=c15,
                                           op0=ALU.mult, op1=ALU.add)
            dst = packed32[:, 0:1] if it == 1 else y
            nc.vector.tensor_mul(dst, y, t1)
        rstd = packed32[:, 0:1]
        chan = ot_ps.tile([128, 2], F32, tag="ot")  # expand groups->channels,
        nc.tensor.matmul(chan[0:64, :], lhsT=p32x64, rhs=packed32)  # both halves
        nc.tensor.matmul(chan[64:128, :], lhsT=p32x64, rhs=packed32,
                         tile_position=(0, 64))
        scale_col = tiny.tile([128, 1], F32)      # rstd_g * gamma_c (mirrored)
        nc.vector.tensor_mul(scale_col, chan[:, 0:1], gamma_col)
        # ---- fold GN into projection weights (both halves in one op); the
        # ---- weight scalings gate the projections so they come first ----
        wk_sc = tiny.tile([128, 64], BF16)
        nc.vector.tensor_scalar_mul(wk_sc, wk_sb, scale_col)
        scale_q = tiny.tile([128, 1], F32)        # q path also carries the
        nc.vector.tensor_scalar_mul(scale_q, scale_col, SCALE_A)  # exp slope
        wq_sc = tiny.tile([128, 64], BF16)
        nc.vector.tensor_scalar_mul(wq_sc, wq_sb, scale_q)
        wv_sc = tiny.tile([128, 64], BF16)
        nc.vector.tensor_scalar_mul(wv_sc, wv_sb, scale_col)
        gnbias = tiny.tile([65, 1], F32)          # beta - mean*scale, aug 1
        nc.vector.tensor_mul(gnbias[0:64, :], chan[0:64, 1:2], scale_col[0:64, :])
        nc.vector.tensor_sub(gnbias[0:64, :], beta_col, gnbias[0:64, :])
        nc.gpsimd.memset(gnbias[64:65, :], 1.0)

        bqp = ot_ps.tile([128, 1], F32, tag="ot")  # total q bias, both halves
        nc.tensor.matmul(bqp[0:64, :], lhsT=wq_aug, rhs=gnbias)
        nc.tensor.matmul(bqp[64:128, :], lhsT=wq_aug, rhs=gnbias,
                         tile_position=(0, 64))
        bq_col = tiny.tile([128, 1], F32)
        nc.vector.tensor_scalar_mul(bq_col, bqp, SCALE_A)
        # bvo row for wo_aug: bvo = (gnbias@Wv + bv) @ wo, bounced through HBM
        # to land on partition 64 (engines are lane-locked; DMA is not). This
        # only gates the first output projection, well off the critical path.
        bvcp = ot_ps.tile([65, 1], F32, tag="ot")
        nc.tensor.matmul(bvcp, lhsT=wv_aug, rhs=gnbias)
        bv_col = tiny.tile([64, 1], F32)
        nc.vector.tensor_copy(bv_col, bvcp[0:64, :])
        bvop = ot_ps.tile([1, 64], F32, tag="ot")
        nc.tensor.matmul(bvop, lhsT=bv_col, rhs=wo_sb)
        bvo_row = tiny.tile([1, 64], F32)
        nc.vector.tensor_copy(bvo_row, bvop)
        bvo_stage = nc.dram_tensor("bvo_stage", [64], F32).ap()
        nc.sync.dma_start(out=bvo_stage.rearrange("(o c) -> o c", o=1), in_=bvo_row)
        nc.gpsimd.dma_start(out=wo_aug[64:65, 0:64],
                            in_=bvo_stage.rearrange("(o c) -> o c", o=1))

        # ---- q/k projections -> fp8 DoubleRow operands ----
        # The quads project exactly as in the bf16 version (channels on
        # partitions, both mirror halves), drain straight to fp8, and four
        # SBUF-SBUF DMAs per quad refold each tensor into the DoubleRow block
        # layout [32 partitions (channel pair), 2 k-tiles, cols]: partition p,
        # tile t <-> channel 32t+p. Key chunks are spread over FOUR 32-row
        # array quadrants (rows 32q:32q+32 <-> chunk octet 8q:8q+8) so four
        # score matmuls can ride the PE concurrently; qT_dr carries all
        # queries on every quadrant (qs8's hi half is already the mirror the
        # B-stream quadrants need). Drains are split across DVE and ACT.
        ks8 = big.tile([128, SQ], F8)
        qs8 = big.tile([128, SQ], F8)
        kT_dr = big.tile([128, 2, 1024], F8)
        qT_dr = big.tile([128, 2, SQ], F8)

        def kq_quad(dst, w_sc, lo_cols, hi_cols, bias):
            g = st_ps.tile([128, 1024], F32, tag="st")
            nc.tensor.matmul(g[0:64, 0:512], lhsT=w_sc[0:64, :],
                             rhs=xT[0:64, lo_cols:lo_cols + 512],
                             tile_position=(0, 0))
            nc.tensor.matmul(g[64:128, 512:1024], lhsT=w_sc[64:128, :],
                             rhs=xT[64:128, hi_cols + 512:hi_cols + 1024],
                             tile_position=(64, 64))
            nc.tensor.matmul(g[64:128, 0:512], lhsT=w_sc[64:128, :],
                             rhs=xT[64:128, hi_cols:hi_cols + 512],
                             tile_position=(64, 64))
            nc.tensor.matmul(g[0:64, 512:1024], lhsT=w_sc[0:64, :],
                             rhs=xT[0:64, lo_cols + 512:lo_cols + 1024],
                             tile_position=(0, 0))
            if bias is None:
                nc.scalar.copy(out=dst[:, 0:512], in_=g[:, 0:512])
                nc.vector.tensor_copy(dst[:, 512:1024], g[:, 512:1024])
            else:
                nc.vector.tensor_scalar_add(dst[:, 0:512], g[:, 0:512], bias)
                nc.scalar.add(out=dst[:, 512:1024], in_=g[:, 512:1024],
                              add=bias)

        rings = [nc.sync, nc.gpsimd, nc.scalar]

        def remap_k(quad_cols):
            # ks8 rows 0:64 = chunks 0:15, rows 64:128 = chunks 16:31; the
            # column quad picks the even or odd octet of each.
            qa = 0 if quad_cols == 0 else 1      # quadrants Q0/Q2 or Q1/Q3
            cs = slice(quad_cols, quad_cols + 1024)
            for n, (src, dst) in enumerate(((0, 32 * qa), (64, 64 + 32 * qa))):
                for t in range(2):
                    rings[(2 * n + t) % 3].dma_start(
                        out=kT_dr[dst:dst + 32, t, 0:1024],
                        in_=ks8[src + 32 * t:src + 32 * t + 32, cs])

        def remap_q(quad_cols):
            cs = slice(quad_cols, quad_cols + 1024)
            for n, (src, dst) in enumerate(((0, 0), (0, 32), (64, 64),
                                            (64, 96))):
                for t in range(2):
                    rings[(2 * n + t) % 3].dma_start(
                        out=qT_dr[dst:dst + 32, t, cs],
                        in_=qs8[src + 32 * t:src + 32 * t + 32, cs])

        kq_quad(ks8[:, 0:1024], wk_sc, 0, 2048, None)
        remap_k(0)
        kq_quad(ks8[:, 1024:2048], wk_sc, 1024, 3072, None)
        remap_k(1024)
        kq_quad(qs8[:, 0:1024], wq_sc, 0, 0, bq_col)
        remap_q(0)

        # v in natural [t, c] fp8 layout; groups of 4 chunks {p, 8+p, 16+p,
        # 24+p}: lo-rows compute the a=0 chunks, hi-rows the a=1 chunks, into
        # two one-bank tiles (concurrent row-tiles drain into distinct banks).
        # Column 64 = ones (exact in e4m3) for the softmax denominator.
        v_big = big.tile([128, N_CHUNK, 65], F8)
        nc.gpsimd.memset(v_big[:, :, 64:65], 1.0)
        v4 = v_big.rearrange("q (a b g) c -> q a b g c", a=2, b=2)

        def v_group(p):
            vga = ot_ps.tile([128, 2, 64], F32, tag="ot")
            vgb = ot_ps.tile([128, 2, 64], F32, tag="ot")
            for a, vg in ((0, vga), (1, vgb)):
                half = slice(64, 128) if a else slice(0, 64)
                tp = (64, 0) if a else (0, 0)
                for b in range(2):
                    ch = a * 16 + b * 8 + p
                    nc.tensor.matmul(vg[:, b, :],
                                     lhsT=xT[half, 128 * ch:128 * (ch + 1)],
                                     rhs=wv_sc[half, :], tile_position=tp)
            nc.vector.tensor_copy(v4[:, 0, :, p, 0:64], vga)
            nc.vector.tensor_copy(v4[:, 1, :, p, 0:64], vgb)

        for p in range(8):
            v_group(p)

        # stripe 0/1 only need q columns 0:1024; the second q quad projects
        # under the v groups and the first attention iterations.
        kq_quad(qs8[:, 1024:2048], wq_sc, 1024, 1024, bq_col)
        remap_q(1024)

        # ---- residual base: x + bo (needed only by epilogues) ----
        xq_sb = big.tile([128, NQ, 64], F32)
        nc.sync.dma_start(out=xq_sb, in_=x_q.rearrange("(m p) c -> p m c", p=128))
        xb_sb = big.tile([128, NQ, 64], F32)
        nc.vector.tensor_add(xb_sb, xq_sb,
                             bo_bcast.rearrange("p (o c) -> p o c", o=1).broadcast_to([128, NQ, 64]))

        # ---- main attention loop ----
        p_pool = ctx.enter_context(tc.tile_pool(name="p_pool", bufs=6))
        ep_pool = ctx.enter_context(tc.tile_pool(name="ep_pool", bufs=3))
        N_PAIR = N_CHUNK // 2

        def emit_o(io, ot, pt):
            # chunks arrive as 0, 16, 8, 24, 1, ...: first is 0, last is 31
            first = io == 0
            last = io == N_CHUNK - 1
            nc.tensor.matmul(ot, lhsT=v_big[:, io, :], rhs=pt,
                             start=first, stop=last)

        def make_epilogue(j, ot_sb):
            last_stripe = j == N_STRIPE - 1

            def epi():
                res = ep_pool.tile([128, 4, 64], F32, tag="res", bufs=2)
                op4 = ot_ps.tile([128, 4, 128], F32, tag="ot")
                for m in range(4):
                    nc.tensor.matmul(op4[:, m, 0:65],
                                     lhsT=ot_sb[:, 128 * m:128 * (m + 1)],
                                     rhs=wo_aug)
                    rl = ep_pool.tile([128, 1], F32, tag="rl")
                    nc.vector.reciprocal(rl, op4[:, m, 64:65])
                    nc.vector.scalar_tensor_tensor(out=res[:, m, :],
                                                   in0=op4[:, m, 0:64],
                                                   scalar=rl,
                                                   in1=xb_sb[:, 4 * j + m, :],
                                                   op0=ALU.mult, op1=ALU.add)
                    if last_stripe:
                        # tail latency: ship each chunk on its own ring so the
                        # ~650ns per-issue cost doesn't serialize (the exit
                        # protocol's round-2 stagger is fixed, not DMA-gated)
                        base = 512 * j + 128 * m
                        ring = [nc.sync, nc.scalar, nc.sync, nc.scalar][m]
                        ring.dma_start(out=out_d[base:base + 128, :],
                                       in_=res[:, m, :])
                if not last_stripe:
                    nc.sync.dma_start(
                        out=out_d[512 * j:512 * (j + 1), :].rearrange("(m p) c -> p m c", p=128),
                        in_=res)
            return epi

        pending_epilogue = None
        for j in range(N_STRIPE):
            ot = ot_ps.tile([65, 512], F32, tag="ot")
            pts = {}
            qc = slice(512 * j, 512 * (j + 1))

            def score_exp(i):
                # iteration i scores chunks i, 8+i, 16+i, 24+i: one DR matmul
                # per 32-row quadrant, all four riding the array together.
                kc = slice(128 * i, 128 * (i + 1))
                st2a = st_ps.tile([128, 1024], F32, tag="st")
                nc.tensor.matmul(st2a[:, 0:512], lhsT=kT_dr[0:32, :, kc],
                                 rhs=qT_dr[0:32, :, qc],
                                 perf_mode=PM.DoubleRow, tile_position=(0, 0))
                nc.tensor.matmul(st2a[:, 512:1024], lhsT=kT_dr[64:96, :, kc],
                                 rhs=qT_dr[64:96, :, qc],
                                 perf_mode=PM.DoubleRow, tile_position=(64, 0))
                st2b = st_ps.tile([128, 1024], F32, tag="st")
                nc.tensor.matmul(st2b[:, 0:512], lhsT=kT_dr[32:64, :, kc],
                                 rhs=qT_dr[32:64, :, qc],
                                 perf_mode=PM.DoubleRow, tile_position=(32, 0))
                nc.tensor.matmul(st2b[:, 512:1024], lhsT=kT_dr[96:128, :, kc],
                                 rhs=qT_dr[96:128, :, qc],
                                 perf_mode=PM.DoubleRow, tile_position=(96, 0))
                pta = p_pool.tile([128, 1024], F8, tag="p")
                nc.scalar.activation(pta, st2a, AF.Exp, bias=0.0,
                                     scale=1.0 / A_EXP)
                if i in ACT_B_ITERS:
                    ptb = p_pool.tile([128, 1024], F8, tag="p")
                    nc.scalar.activation(ptb, st2b, AF.Exp, bias=0.0,
                                         scale=1.0 / A_EXP)
                    pts[i] = (pta, ptb)
                else:
                    ptb = p_pool.tile([128, 1024], I8, tag="p")
                    nc.vector.tensor_scalar_add(ptb, st2b, B_EXP)
                    pts[i] = (pta, ptb.bitcast(F8))

            # att@v trails its exp by one iteration so the in-order PE queue
            # never parks on an exp still in flight; the last iteration
            # flushes after the loop.
            for i in range(N_PAIR // 2 + 1):
                if i < N_PAIR // 2:
                    score_exp(i)
                if i == 2 and pending_epilogue is not None:
                    pending_epilogue()
                    pending_epilogue = None
                io = i - 1
                if io >= 0:
                    pta, ptb = pts.pop(io)
                    emit_o(io, ot, pta[:, 0:512])
                    emit_o(16 + io, ot, pta[:, 512:1024])
                    emit_o(8 + io, ot, ptb[:, 0:512])
                    emit_o(24 + io, ot, ptb[:, 512:1024])
            # drain the accumulator (+ l row) to SBUF. The last stripe drains
            # per column-chunk so its epilogue (on the critical tail) starts
            # as soon as the first chunk lands.
            ot_sb = ep_pool.tile([65, 512], BF16, bufs=2, tag="ot_sb")
            if j == N_STRIPE - 1:
                for m in range(4):
                    cs = slice(128 * m, 128 * (m + 1))
                    if m % 2 == 0:
                        nc.vector.tensor_copy(ot_sb[:, cs], ot[:, cs])
                    else:
                        nc.scalar.copy(out=ot_sb[:, cs], in_=ot[:, cs])
            else:
                nc.scalar.copy(out=ot_sb[:, 0:256], in_=ot[:, 0:256])
                nc.vector.tensor_copy(ot_sb[:, 256:512], ot[:, 256:512])
            pending_epilogue = make_epilogue(j, ot_sb)
        pending_epilogue()


_NC_CACHE = {}


def _get_nc():
    if "nc" not in _NC_CACHE:
        _NC_CACHE["nc"] = build_kernel()
    return _NC_CACHE["nc"]


def build_in_maps(x, gamma, beta, wq, bq, wk, wv, bv, wo, bo):
    """Per-core NEFF input dicts plus (batch, rows) scatter info per core."""
    x = np.asarray(x, dtype=np.float32)
    shared = {
        "gamma": np.asarray(gamma, np.float32),
        "beta": np.asarray(beta, np.float32),
        "wq": np.asarray(wq, np.float32), "bq": np.asarray(bq, np.float32),
        "wk": np.asarray(wk, np.float32),
        "wv": np.asarray(wv, np.float32), "bv": np.asarray(bv, np.float32),
        "wo": np.asarray(wo, np.float32), "bo": np.asarray(bo, np.float32),
    }
    xf = x.reshape(B, S, C)
    in_maps = []
    scatter = []
    for core in range(8):
        b, h = core // 2, core % 2
        own = slice(h * SQ, (h + 1) * SQ)
        other = slice((1 - h) * SQ, (2 - h) * SQ)
        x_local = np.concatenate([xf[b][own], xf[b][other]], axis=0)
        in_maps.append({
            "xT": np.ascontiguousarray(x_local.T).astype(ml_dtypes.bfloat16),
            "x_q": np.ascontiguousarray(x_local[:SQ]),
            **shared,
        })
        scatter.append((b, np.arange(h * SQ, (h + 1) * SQ)))
    return in_maps, scatter


def _run(in_maps, scatter, **spmd_kwargs):
    nc = _get_nc()
    res = run_bass_kernel_spmd(nc, in_maps, core_ids=list(range(8)),
                               **spmd_kwargs)
    out = np.empty((B, S, C), np.float32)
    for core in range(8):
        b, rows = scatter[core]
        out[b][rows] = res.results[core]["out"]
    return out.reshape(B, H, W, C), res


def kernel(x, gamma, beta, wq, bq, wk, bk, wv, bv, wo, bo):
    # bk is provably a no-op: it shifts each query's scores by the constant
    # bk.q which softmax cancels, so it is not shipped to the device.
    in_maps, scatter = build_in_maps(x, gamma, beta, wq, bq, wk, wv, bv, wo, bo)
    out, _ = _run(in_maps, scatter)
    return out
